# revision 7
# baseline (speedup 1.0000x reference)
"""Trainium2 Bass kernel for nn_Loss_20993800143146 (loss_fn).

Computes, over 8 NeuronCores (data-parallel over batch / bh):
    mel_loss  = mean(|mels_pred * mask - mels_target|)           (mean over full tensor)
    stop_loss = sum(-5 * clamp(log(stop_pred[b, last_idx_b]), -100)) / mask.sum()
    dc        = sum(alignments * band[s,t] * bmask[b]) / (H * lengths.sum() * N)
    out       = mel_loss + stop_loss - 1e-4 * dc

Key algebraic fact: band[s,t] = (s >= clip(5t-50,0,160)) & (s < clip(5t+50,0,160))
is identically zero for t >= 42 (clip hits s=160), so only alignments[:,:,:,:42]
is ever read (~5 MB of the 98 MB tensor).

Sharding: batch dim (16 -> 2 per core) for lengths/mask/stop/mels, bh dim
(64 -> 8 per core) for alignments. Each core reduces its shard to 5 scalars
on-device ([dc_w, mel_num, stop_logp, mask_cnt, len_sum]); the host sums the
8 partial vectors and applies the final constant-denominator arithmetic.
"""

import numpy as np

# Problem constants (hardcoded per contract; kernel.py must be self-contained).
H = 4
B = 16
T = 800
NMEL = 80
S = 160
N = 3
BW = 50
K = T // S  # 5
TC = 42  # band[:, t] == 0 for all t >= TC
NCORES = 8

MEL_ROWS = 2 * T            # 1600 (b,t) rows per core
MEL_PAD_ROWS = 1664         # pad to 128 * 13
MEL_G = 13                  # 80-col groups per partition
ALN_F = N * S * TC // 16    # 1260 free elems per partition (8 b * 16 part/b)

_CACHE = {}


def _band():
    tr = np.arange(TC)
    mn = np.clip(K * tr - BW, 0, S)
    mx = np.clip(K * tr + BW, 0, S)
    rows = np.arange(S)
    return ((rows[:, None] >= mn[None, :]) & (rows[:, None] < mx[None, :]))


def _wband_u8():
    """Band weight tile [128, 1260]: partition p holds rows (p%16)*30+j of the
    (n, s) x t[:TC] block of one b; weight depends only on s = row % 160."""
    band = _band()  # [S, TC] bool
    p_idx = np.arange(128)
    j_idx = np.arange(30)
    s_of = (((p_idx[:, None] % 16) * 30) + j_idx[None, :]) % S  # [128, 30]
    return band[s_of].reshape(128, ALN_F).astype(np.uint8)


def _build_bass():
    import concourse.bass as bass
    import concourse.bacc as bacc
    import concourse.tile as tile
    import concourse.mybir as mybir
    from contextlib import ExitStack

    f32 = mybir.dt.float32
    u8 = mybir.dt.uint8
    i32 = mybir.dt.int32
    Alu = mybir.AluOpType
    Act = mybir.ActivationFunctionType
    Ax = mybir.AxisListType

    nc = bacc.Bacc("TRN2", target_bir_lowering=False, debug=False,
                   num_devices=NCORES)

    melsp = nc.dram_tensor("melsp", [128, MEL_G * NMEL], f32, kind="ExternalInput").ap()
    melst = nc.dram_tensor("melst", [128, MEL_G * NMEL], f32, kind="ExternalInput").ap()
    mask13 = nc.dram_tensor("mask13", [128, MEL_G], u8, kind="ExternalInput").ap()
    masks = nc.dram_tensor("masks", [2, T], u8, kind="ExternalInput").ap()
    stop = nc.dram_tensor("stop", [2, T], f32, kind="ExternalInput").ap()
    iotap1 = nc.dram_tensor("iotap1", [2, T], f32, kind="ExternalInput").ap()
    align = nc.dram_tensor("align", [128, ALN_F], f32, kind="ExternalInput").ap()
    wband = nc.dram_tensor("wband", [128, ALN_F], u8, kind="ExternalInput").ap()
    lenrep = nc.dram_tensor("lenrep", [128, 1], i32, kind="ExternalInput").ap()
    lenfull = nc.dram_tensor("lenfull", [1, B], i32, kind="ExternalInput").ap()
    ones = nc.dram_tensor("ones", [128, 1], f32, kind="ExternalInput").ap()
    out = nc.dram_tensor("out", [5, 1], f32, kind="ExternalOutput").ap()

    with tile.TileContext(nc) as tc:
        with ExitStack() as ctx:
            pool = ctx.enter_context(tc.tile_pool(name="main", bufs=1))
            ppool = ctx.enter_context(tc.tile_pool(name="ps", bufs=1, space="PSUM"))

            # ---- loads ----
            mp_t = pool.tile([128, MEL_G * NMEL], f32, tag="mp")
            nc.sync.dma_start(mp_t[:], melsp)
            mt_t = pool.tile([128, MEL_G * NMEL], f32, tag="mt")
            nc.sync.dma_start(mt_t[:], melst)
            m13_t = pool.tile([128, MEL_G], u8, tag="m13")
            nc.sync.dma_start(m13_t[:], mask13)
            msk_t = pool.tile([2, T], u8, tag="msk")
            nc.sync.dma_start(msk_t[:], masks)
            stp_t = pool.tile([2, T], f32, tag="stp")
            nc.sync.dma_start(stp_t[:], stop)
            io_t = pool.tile([2, T], f32, tag="io")
            nc.sync.dma_start(io_t[:], iotap1)
            al_t = pool.tile([128, ALN_F], f32, tag="al")
            nc.sync.dma_start(al_t[:], align)
            wb_t = pool.tile([128, ALN_F], u8, tag="wb")
            nc.sync.dma_start(wb_t[:], wband)
            lr_t = pool.tile([128, 1], i32, tag="lr")
            nc.sync.dma_start(lr_t[:], lenrep)
            lf_t = pool.tile([1, B], i32, tag="lf")
            nc.sync.dma_start(lf_t[:], lenfull)
            on_t = pool.tile([128, 1], f32, tag="on")
            nc.sync.dma_start(on_t[:], ones)

            # stats[:, c]: 0 = dc_w, 1 = mel_num, 2 = stop logp, 3 = mask cnt,
            # 4 = lengths sum (row 0 only)
            st_t = pool.tile([128, 5], f32, tag="st")
            nc.vector.memset(st_t[:], 0.0)

            # ---- mel term ----
            d_t = pool.tile([128, MEL_G * NMEL], f32, tag="d")
            nc.vector.tensor_sub(d_t[:], mp_t[:], mt_t[:])
            v1_t = pool.tile([128, MEL_G], f32, tag="v1")
            nc.vector.tensor_reduce(
                v1_t[:], d_t[:].rearrange("p (g m) -> p g m", m=NMEL),
                axis=Ax.X, op=Alu.add, apply_absolute_value=True)
            v2_t = pool.tile([128, MEL_G], f32, tag="v2")
            nc.vector.tensor_reduce(
                v2_t[:], mt_t[:].rearrange("p (g m) -> p g m", m=NMEL),
                axis=Ax.X, op=Alu.add, apply_absolute_value=True)
            m13f_t = pool.tile([128, MEL_G], f32, tag="m13f")
            nc.vector.tensor_copy(m13f_t[:], m13_t[:])

            w1_t = pool.tile([128, MEL_G], f32, tag="w1")
            c1_t = pool.tile([128, 1], f32, tag="c1")
            nc.vector.scalar_tensor_tensor(
                w1_t[:], v1_t[:], 1.0, m13f_t[:],
                op0=Alu.bypass, op1=Alu.mult, accum_out=c1_t[:])
            c2_t = pool.tile([128, 1], f32, tag="c2")
            nc.vector.tensor_reduce(c2_t[:], v2_t[:], axis=Ax.X, op=Alu.add)
            w2_t = pool.tile([128, MEL_G], f32, tag="w2")
            c3_t = pool.tile([128, 1], f32, tag="c3")
            nc.vector.scalar_tensor_tensor(
                w2_t[:], v2_t[:], 1.0, m13f_t[:],
                op0=Alu.bypass, op1=Alu.mult, accum_out=c3_t[:])
            c12_t = pool.tile([128, 1], f32, tag="c12")
            nc.vector.tensor_add(c12_t[:], c1_t[:], c2_t[:])
            nc.vector.tensor_sub(st_t[:, 1:2], c12_t[:], c3_t[:])
            # mask count partial
            nc.vector.tensor_reduce(st_t[:, 3:4], m13f_t[:], axis=Ax.X, op=Alu.add)

            # ---- stop term ----
            msf_t = pool.tile([2, T], f32, tag="msf")
            nc.vector.tensor_copy(msf_t[:], msk_t[:])
            tl_t = pool.tile([2, T], f32, tag="tl")
            nc.vector.tensor_mul(tl_t[:], io_t[:], msf_t[:])
            mx_t = pool.tile([2, 1], f32, tag="mx")
            nc.vector.tensor_reduce(mx_t[:], tl_t[:], axis=Ax.X, op=Alu.max)
            eq_t = pool.tile([2, T], f32, tag="eq")
            pl_t = pool.tile([2, 1], f32, tag="pl")
            nc.vector.scalar_tensor_tensor(
                eq_t[:], tl_t[:], mx_t[:], stp_t[:],
                op0=Alu.is_equal, op1=Alu.mult, accum_out=pl_t[:])
            lg_t = pool.tile([2, 1], f32, tag="lg")
            nc.scalar.activation(lg_t[:], pl_t[:], Act.Ln)
            nc.vector.tensor_scalar_max(st_t[0:2, 2:3], lg_t[:], -100.0)

            # ---- lengths sum + bmask ----
            lff_t = pool.tile([1, B], f32, tag="lff")
            nc.vector.tensor_copy(lff_t[:], lf_t[:])
            nc.vector.tensor_reduce(st_t[0:1, 4:5], lff_t[:], axis=Ax.X, op=Alu.add)
            lrf_t = pool.tile([128, 1], f32, tag="lrf")
            nc.vector.tensor_copy(lrf_t[:], lr_t[:])
            bm_t = pool.tile([128, 1], f32, tag="bm")
            nc.vector.tensor_scalar(bm_t[:], lrf_t[:], float(T), None, op0=Alu.is_le)

            # ---- dc term ----
            wf_t = pool.tile([128, ALN_F], f32, tag="wf")
            nc.vector.tensor_copy(wf_t[:], wb_t[:])
            pr_t = pool.tile([128, ALN_F], f32, tag="pr")
            dcpp_t = pool.tile([128, 1], f32, tag="dcpp")
            nc.vector.scalar_tensor_tensor(
                pr_t[:], al_t[:], 1.0, wf_t[:],
                op0=Alu.bypass, op1=Alu.mult, accum_out=dcpp_t[:])
            nc.vector.tensor_mul(st_t[:, 0:1], dcpp_t[:], bm_t[:])

            # ---- partition reduction via PE: out[5,1] = stats.T @ ones ----
            pt = ppool.tile([5, 1], f32, tag="pt")
            nc.tensor.matmul(pt[:], lhsT=st_t[:], rhs=on_t[:],
                             start=True, stop=True)
            ex_t = pool.tile([5, 1], f32, tag="ex")
            nc.vector.tensor_copy(ex_t[:], pt[:])
            nc.sync.dma_start(out, ex_t[:])

    nc.compile()
    return nc


def _get_nc():
    if "nc" not in _CACHE:
        _CACHE["nc"] = _build_bass()
    return _CACHE["nc"]


def make_in_maps(lengths, mask, stop_pred, mels_pred, mels_target, alignments):
    """Shard full inputs into the 8 per-core input dicts."""
    lengths = np.ascontiguousarray(lengths, dtype=np.int32)
    mask_u8 = np.ascontiguousarray(mask).view(np.uint8) if mask.dtype == np.bool_ \
        else np.ascontiguousarray(mask.astype(np.uint8))
    stop_pred = np.ascontiguousarray(stop_pred, dtype=np.float32)
    mels_pred = np.ascontiguousarray(mels_pred, dtype=np.float32)
    mels_target = np.ascontiguousarray(mels_target, dtype=np.float32)
    alignments = np.ascontiguousarray(alignments, dtype=np.float32)

    wband = _wband_u8()
    iotap1 = np.broadcast_to(
        np.arange(1, T + 1, dtype=np.float32)[None, :], (2, T)).copy()
    ones = np.ones((128, 1), dtype=np.float32)
    lenfull = lengths.reshape(1, B)

    def pad_rows(x2d, cols):
        padded = np.zeros((MEL_PAD_ROWS, cols), x2d.dtype)
        padded[:MEL_ROWS] = x2d
        return padded

    in_maps = []
    for c in range(NCORES):
        bs = slice(2 * c, 2 * c + 2)
        mp = pad_rows(mels_pred[bs].reshape(MEL_ROWS, NMEL), NMEL).reshape(128, MEL_G * NMEL)
        mt = pad_rows(mels_target[bs].reshape(MEL_ROWS, NMEL), NMEL).reshape(128, MEL_G * NMEL)
        m13 = pad_rows(mask_u8[bs].reshape(MEL_ROWS, 1), 1).reshape(128, MEL_G)
        aln = np.ascontiguousarray(
            alignments[:, 8 * c:8 * c + 8, :, :TC].transpose(1, 0, 2, 3)
        ).reshape(128, ALN_F)
        b_lo = 8 * (c % 2)
        lenrep = np.repeat(lengths[b_lo:b_lo + 8], 16).reshape(128, 1)
        in_maps.append({
            "melsp": mp, "melst": mt, "mask13": m13,
            "masks": mask_u8[bs], "stop": stop_pred[bs], "iotap1": iotap1,
            "align": aln, "wband": wband,
            "lenrep": lenrep, "lenfull": lenfull, "ones": ones,
        })
    return in_maps


def combine_partials(partials):
    """partials: list of 8 arrays [5,1] -> final scalar (0-d f32 ndarray)."""
    ps = np.stack([np.asarray(p, dtype=np.float64).reshape(5) for p in partials])
    dc_w = ps[:, 0].sum()
    mel_num = ps[:, 1].sum()
    logp = ps[:, 2].sum()
    mask_cnt = ps[:, 3].sum()
    len_sum = ps[0, 4]
    mel_loss = mel_num / float(B * T * NMEL)
    stop_loss = -5.0 * logp / mask_cnt
    dc = dc_w / (H * len_sum * N)
    return np.array(np.float32(mel_loss + stop_loss - 1e-4 * dc))


def kernel(lengths, mask, stop_pred, mels_pred, mels_target, alignments):
    from concourse.bass_utils import run_bass_kernel_spmd

    nc = _get_nc()
    in_maps = make_in_maps(lengths, np.asarray(mask), stop_pred,
                           mels_pred, mels_target, alignments)
    res = run_bass_kernel_spmd(nc, in_maps, list(range(NCORES)))
    return combine_partials([r["out"] for r in res.results])
